# revision 26
# baseline (speedup 1.0000x reference)
"""AttentiveTransformer (Dense + BN(eval) + prior-scale + sparsemax) on 8 TRN2 cores.

Math per row (B=131072 rows, data-parallel over 8 cores):
    y   = x @ (W * bn_inv) + (bn_bias - bn_mean * bn_inv)   # BN folded into W/bias
    z   = y * priors
    out = sparsemax(z)          # row-wise, D=256, support capped at top-8

v4 pipeline — bf16 x path so the x^T repartition runs on the DMA XBAR
instead of the PE/ACT engines:
    GPS  : x loads as casting DMAs (f32 HBM -> bf16 SBUF, software DGE),
           out stores, iota/msel muls
    SYNC : one DMA-XBAR transpose per 128-row tile (bf16 16-bit path),
           priors loads
    PE   : 4 bf16 matmuls per tile (bf16 LDWEIGHTS at half cost), f32 PSUM
    DVE  : z = y*priors (PSUM read), top-8 via max8, prefix-sum tau0 math
    ACT  : out = relu(z - tau0), one activation per tile

Sharding: pure data-parallel on the batch dim; W/BN replicated per core.
"""

import numpy as np

import concourse.mybir as mybir
import concourse.tile as tile
from concourse import bacc
from concourse.bass_utils import run_bass_kernel_spmd

F32 = mybir.dt.float32
FP16 = mybir.dt.float16
Alu = mybir.AluOpType
Act = mybir.ActivationFunctionType

NCORES = 8
B = 131072
DIN = 512
DOUT = 256
P = 128
BC = B // NCORES            # rows per core (16384)
G = 8                       # row-tiles per super-batch
TILES = BC // P             # row-tiles per core (128)
NBATCH = TILES // G         # super-batches per core (16)
KC = DIN // P               # K chunks (4)
K8 = 8

BN_EPS = 1e-5

_CACHE = {}
LAST_RESULTS = None


def _build(use_bias):
    nc = bacc.Bacc("TRN2", target_bir_lowering=False, debug=False)

    x_d = nc.dram_tensor("x", [BC, DIN], F32, kind="ExternalInput").ap()
    pri_d = nc.dram_tensor("priors", [BC, DOUT], F32, kind="ExternalInput").ap()
    w_d = nc.dram_tensor("w", [DIN, DOUT], F32, kind="ExternalInput").ap()
    b_d = nc.dram_tensor("b", [1, DOUT], F32, kind="ExternalInput").ap()
    iota_d = nc.dram_tensor("iota8", [P, G * K8], F32, kind="ExternalInput").ap()
    out_d = nc.dram_tensor("out", [BC, DOUT], F32, kind="ExternalOutput").ap()

    xg = x_d.rearrange("(g p t) d -> g p t d", p=P, t=G)
    pg = pri_d.rearrange("(g p t) d -> g p t d", p=P, t=G)
    og = out_d.rearrange("(g p t) d -> g p t d", p=P, t=G)

    with tile.TileContext(nc) as tc:
        with (
            tc.tile_pool(name="static", bufs=1) as sp,
            tc.tile_pool(name="xin", bufs=3) as xp,
            tc.tile_pool(name="xh", bufs=4) as xhp,
            tc.tile_pool(name="pin", bufs=3) as pp,
            tc.tile_pool(name="oout", bufs=3) as op_,
            tc.tile_pool(name="zb", bufs=3) as zp,
            tc.tile_pool(name="xt", bufs=8) as xtp,
            tc.tile_pool(name="small", bufs=3) as smp,
            tc.tile_pool(name="psy", bufs=6, space="PSUM") as psy,
        ):
            # ---- statics (weights on the scalar HWDGE queue) ----
            w_sb = sp.tile([P, KC, DOUT], F32)
            nc.scalar.dma_start(w_sb, w_d.rearrange("(c p) n -> p c n", p=P))
            wb_sb = sp.tile([P, KC, DOUT], FP16)
            nc.vector.tensor_copy(wb_sb, w_sb)

            if use_bias:
                b_sb = sp.tile([1, DOUT], F32)
                nc.scalar.dma_start(b_sb, b_d)
                bb_sb = sp.tile([1, DOUT], FP16)
                nc.vector.tensor_copy(bb_sb, b_sb)
                ones_sb = sp.tile([1, P], FP16)
                nc.vector.memset(ones_sb, 1.0)

            iota_sb = sp.tile([P, G * K8], F32)
            nc.scalar.dma_start(iota_sb, iota_d)

            keep_sb = sp.tile([P, G * K8], F32)
            nc.vector.memset(keep_sb, 1.0)
            nc.vector.memset(
                keep_sb.rearrange("p (g s) -> p g s", s=K8)[:, :, 0:1], 0.0
            )

            x_bufs = {}
            xh_bufs = {}
            p_bufs = {}

            def load_x(gi):
                # Hybrid x load: tiles 0..3 arrive as fp16 straight from the
                # casting DMA (software DGE, runs in parallel at ~85 GB/s);
                # tiles 4..7 arrive f32 on the fast HW queue and are
                # converted to fp16 by ACT.
                xh_bufs[gi] = xhp.tile([P, G, DIN], FP16, name="xhbuf")
                nc.gpsimd.dma_start(xh_bufs[gi][:, 0:4, :], xg[gi][:, 0:4, :])
                x_bufs[gi] = xp.tile([P, G // 2, DIN], F32, name="xbuf")
                nc.sync.dma_start(x_bufs[gi], xg[gi][:, 4:8, :])

            def conv_x(gi):
                xb = x_bufs.pop(gi)
                for c in range(0, 4, 2):
                    nc.scalar.copy(
                        xh_bufs[gi][:, 4 + c : 6 + c, :], xb[:, c : c + 2, :]
                    )

            def load_p(gi):
                p_bufs[gi] = pp.tile([P, G, DOUT], F32, name="pbuf")
                nc.sync.dma_start(p_bufs[gi], pg[gi])

            for g in range(NBATCH):
                # Prefetch two super-batches ahead; convert one ahead.
                if g == 0:
                    load_x(0)
                    load_p(0)
                    load_x(1)
                    load_p(1)
                    conv_x(0)
                if g + 2 < NBATCH:
                    load_x(g + 2)
                    load_p(g + 2)
                if g + 1 < NBATCH:
                    conv_x(g + 1)
                x_buf = xh_bufs.pop(g)
                p_buf = p_bufs.pop(g)

                z_buf = zp.tile([P, G, DOUT], F32)
                m8 = smp.tile([P, G, K8], F32, tag="m8")
                out_buf = op_.tile([P, G, DOUT], F32)

                # ---- stage A: XBAR-transpose skewed one tile ahead of the
                #      matmuls so the PE never waits on repartitioning ----
                xt_list = [None] * (G // 4)
                y2 = None
                for t in range(G + 4):
                    if t % 4 == 0 and t < G:
                        h = t // 4
                        xt4 = xtp.tile([P, 4, KC, P], FP16, name="xt4")
                        nc.sync.dma_start(
                            xt4, x_buf[:, t : t + 4, :], transpose=True
                        )
                        xt_list[h] = xt4
                    if t >= 4:
                        tt = t - 4
                        if tt % 2 == 0:
                            y2 = psy.tile([P, 2, DOUT], F32)
                        for k in range(KC):
                            nc.tensor.matmul(
                                y2[:, tt % 2, :],
                                xt_list[tt // 4][:, tt % 4, k, :],
                                wb_sb[:, k, :],
                                start=(k == 0),
                                stop=(k == KC - 1) and not use_bias,
                            )
                        if use_bias:
                            nc.tensor.matmul(
                                y2[:, tt % 2, :], ones_sb, bb_sb, start=False, stop=True
                            )
                        if tt % 2 == 1:
                            nc.vector.tensor_mul(
                                z_buf[:, tt - 1 : tt + 1, :],
                                y2,
                                p_buf[:, tt - 1 : tt + 1, :],
                            )
                            nc.vector.max(m8[:, tt - 1, :], z_buf[:, tt - 1, :])
                            nc.vector.max(m8[:, tt, :], z_buf[:, tt, :])

                # ---- stage B: tau0 from top-8 prefix (DVE + GPS) ----
                mflat = m8.rearrange("p g s -> p (g s)")
                cum = smp.tile([P, G * K8], F32, tag="cum")
                nc.vector.tensor_tensor_scan(
                    out=cum,
                    data0=keep_sb,
                    data1=mflat,
                    initial=0.0,
                    op0=Alu.mult,
                    op1=Alu.add,
                )
                jm = smp.tile([P, G * K8], F32, tag="jm")
                nc.gpsimd.tensor_mul(jm, mflat, iota_sb)
                cm1 = smp.tile([P, G * K8], F32, tag="cm1")
                nc.vector.tensor_scalar_sub(cm1, cum, 1.0)
                mask = smp.tile([P, G * K8], F32, tag="mask")
                nc.vector.tensor_tensor(out=mask, in0=jm, in1=cm1, op=Alu.is_gt)
                msel = smp.tile([P, G * K8], F32, tag="msel")
                nc.gpsimd.tensor_mul(msel, mflat, mask)

                s8 = smp.tile([P, G], F32, tag="s8")
                nc.vector.reduce_sum(
                    s8,
                    msel.rearrange("p (g s) -> p g s", s=K8),
                    axis=mybir.AxisListType.X,
                )
                k8 = smp.tile([P, G], F32, tag="k8")
                nc.vector.reduce_sum(
                    k8,
                    mask.rearrange("p (g s) -> p g s", s=K8),
                    axis=mybir.AxisListType.X,
                )
                kr = smp.tile([P, G], F32, tag="kr")
                nc.vector.reciprocal(kr, k8)
                tau0 = smp.tile([P, G], F32, tag="tau0")
                nc.vector.tensor_scalar(
                    out=tau0, in0=s8, scalar1=-1.0, scalar2=None, op0=Alu.add
                )
                nc.vector.tensor_mul(tau0, tau0, kr)
                ntau0 = smp.tile([P, G], F32, tag="ntau0")
                nc.vector.tensor_scalar_mul(ntau0, tau0, -1.0)
                # ---- stage E: out = relu(z - tau0) on ACT, batched store on
                #      the gpsimd queue (ahead of the next x prefetch) ----
                for t in range(G):
                    nc.scalar.activation(
                        out_buf[:, t, :],
                        z_buf[:, t, :],
                        Act.Relu,
                        bias=ntau0[:, t : t + 1],
                    )
                nc.scalar.dma_start(og[g], out_buf)

    nc.compile()
    return nc


def kernel(input_x, priors, W, bn_scale, bn_bias, bn_mean, bn_var):
    global LAST_RESULTS
    input_x = np.ascontiguousarray(input_x, dtype=np.float32)
    priors = np.ascontiguousarray(priors, dtype=np.float32)

    inv = (
        bn_scale.astype(np.float32)
        / np.sqrt(bn_var.astype(np.float32) + np.float32(BN_EPS))
    ).astype(np.float32)
    wf = np.ascontiguousarray(W.astype(np.float32) * inv[None, :])
    bf = np.ascontiguousarray(
        (bn_bias.astype(np.float32) - bn_mean.astype(np.float32) * inv)[None, :]
    )
    use_bias = bool(np.any(bf != 0.0))

    iota8 = np.ascontiguousarray(
        np.tile(np.arange(1, K8 + 1, dtype=np.float32), (P, G))
    )

    key = ("nc", use_bias)
    if key not in _CACHE:
        _CACHE[key] = _build(use_bias)
    nc = _CACHE[key]

    in_maps = []
    for c in range(NCORES):
        in_maps.append(
            {
                "x": input_x[c * BC : (c + 1) * BC],
                "priors": priors[c * BC : (c + 1) * BC],
                "w": wf,
                "b": bf,
                "iota8": iota8,
            }
        )

    res = run_bass_kernel_spmd(nc, in_maps, list(range(NCORES)))
    LAST_RESULTS = res
    out = np.concatenate([res.results[c]["out"] for c in range(NCORES)], axis=0)
    return out


# revision 27
# speedup vs baseline: 1.0501x; 1.0501x over previous
"""AttentiveTransformer (Dense + BN(eval) + prior-scale + sparsemax) on 8 TRN2 cores.

Math per row (B=131072 rows, data-parallel over 8 cores):
    y   = x @ (W * bn_inv) + (bn_bias - bn_mean * bn_inv)   # BN folded into W/bias
    z   = y * priors
    out = sparsemax(z)          # row-wise, D=256, support capped at top-8

v4 pipeline — bf16 x path so the x^T repartition runs on the DMA XBAR
instead of the PE/ACT engines:
    GPS  : x loads as casting DMAs (f32 HBM -> bf16 SBUF, software DGE),
           out stores, iota/msel muls
    SYNC : one DMA-XBAR transpose per 128-row tile (bf16 16-bit path),
           priors loads
    PE   : 4 bf16 matmuls per tile (bf16 LDWEIGHTS at half cost), f32 PSUM
    DVE  : z = y*priors (PSUM read), top-8 via max8, prefix-sum tau0 math
    ACT  : out = relu(z - tau0), one activation per tile

Sharding: pure data-parallel on the batch dim; W/BN replicated per core.
"""

import numpy as np

import concourse.mybir as mybir
import concourse.tile as tile
from concourse import bacc
from concourse.bass_utils import run_bass_kernel_spmd

F32 = mybir.dt.float32
FP16 = mybir.dt.float16
Alu = mybir.AluOpType
Act = mybir.ActivationFunctionType

NCORES = 8
B = 131072
DIN = 512
DOUT = 256
P = 128
BC = B // NCORES            # rows per core (16384)
G = 8                       # row-tiles per super-batch
TILES = BC // P             # row-tiles per core (128)
NBATCH = TILES // G         # super-batches per core (16)
KC = DIN // P               # K chunks (4)
K8 = 8

BN_EPS = 1e-5

_CACHE = {}
LAST_RESULTS = None


def _build(use_bias):
    nc = bacc.Bacc("TRN2", target_bir_lowering=False, debug=False)

    x_d = nc.dram_tensor("x", [BC, DIN], F32, kind="ExternalInput").ap()
    pri_d = nc.dram_tensor("priors", [BC, DOUT], F32, kind="ExternalInput").ap()
    w_d = nc.dram_tensor("w", [DIN, DOUT], F32, kind="ExternalInput").ap()
    b_d = nc.dram_tensor("b", [1, DOUT], F32, kind="ExternalInput").ap()
    iota_d = nc.dram_tensor("iota8", [P, G * K8], F32, kind="ExternalInput").ap()
    out_d = nc.dram_tensor("out", [BC, DOUT], F32, kind="ExternalOutput").ap()

    xg = x_d.rearrange("(g p t) d -> g p t d", p=P, t=G)
    pg = pri_d.rearrange("(g p t) d -> g p t d", p=P, t=G)
    og = out_d.rearrange("(g p t) d -> g p t d", p=P, t=G)

    with tile.TileContext(nc) as tc:
        with (
            tc.tile_pool(name="static", bufs=1) as sp,
            tc.tile_pool(name="xin", bufs=3) as xp,
            tc.tile_pool(name="xh", bufs=4) as xhp,
            tc.tile_pool(name="pin", bufs=3) as pp,
            tc.tile_pool(name="oout", bufs=3) as op_,
            tc.tile_pool(name="zb", bufs=3) as zp,
            tc.tile_pool(name="xt", bufs=8) as xtp,
            tc.tile_pool(name="small", bufs=3) as smp,
            tc.tile_pool(name="psy", bufs=6, space="PSUM") as psy,
        ):
            # ---- statics (weights on the scalar HWDGE queue) ----
            w_sb = sp.tile([P, KC, DOUT], F32)
            nc.scalar.dma_start(w_sb, w_d.rearrange("(c p) n -> p c n", p=P))
            wb_sb = sp.tile([P, KC, DOUT], FP16)
            nc.vector.tensor_copy(wb_sb, w_sb)

            if use_bias:
                b_sb = sp.tile([1, DOUT], F32)
                nc.scalar.dma_start(b_sb, b_d)
                bb_sb = sp.tile([1, DOUT], FP16)
                nc.vector.tensor_copy(bb_sb, b_sb)
                ones_sb = sp.tile([1, P], FP16)
                nc.vector.memset(ones_sb, 1.0)

            iota_sb = sp.tile([P, G * K8], F32)
            nc.scalar.dma_start(iota_sb, iota_d)

            keep_sb = sp.tile([P, G * K8], F32)
            nc.vector.memset(keep_sb, 1.0)
            nc.vector.memset(
                keep_sb.rearrange("p (g s) -> p g s", s=K8)[:, :, 0:1], 0.0
            )

            x_bufs = {}
            xh_bufs = {}
            p_bufs = {}

            def load_x(gi):
                x_bufs[gi] = xp.tile([P, G, DIN], F32, name="xbuf")
                xc = 2 if gi == 0 else 4
                for c in range(0, G, xc):
                    nc.sync.dma_start(
                        x_bufs[gi][:, c : c + xc, :], xg[gi][:, c : c + xc, :]
                    )

            def conv_x(gi):
                # f32 -> fp16 feed for the XBAR transpose. The pool engine's
                # software CAST is slow (~3.6us/2 tiles) but otherwise idle,
                # so it takes half; ACT takes the other half (~1us/2 tiles).
                # Batch 0 goes all-ACT so the pipeline starts fast.
                xb = x_bufs.pop(gi)
                xh_bufs[gi] = xhp.tile([P, G, DIN], FP16, name="xhbuf")
                for c in range(0, G, 2):
                    if c < 4 and gi > 0:
                        nc.gpsimd.tensor_copy(
                            xh_bufs[gi][:, c : c + 2, :], xb[:, c : c + 2, :]
                        )
                    else:
                        nc.scalar.copy(
                            xh_bufs[gi][:, c : c + 2, :], xb[:, c : c + 2, :]
                        )

            def load_p(gi):
                p_bufs[gi] = pp.tile([P, G, DOUT], F32, name="pbuf")
                nc.sync.dma_start(p_bufs[gi], pg[gi])

            for g in range(NBATCH):
                # Prefetch two super-batches ahead; convert one ahead.
                if g == 0:
                    load_x(0)
                    load_p(0)
                    load_x(1)
                    load_p(1)
                    conv_x(0)
                if g + 2 < NBATCH:
                    load_x(g + 2)
                    load_p(g + 2)
                if g + 1 < NBATCH:
                    conv_x(g + 1)
                x_buf = xh_bufs.pop(g)
                p_buf = p_bufs.pop(g)

                z_buf = zp.tile([P, G, DOUT], F32)
                m8 = smp.tile([P, G, K8], F32, tag="m8")
                out_buf = op_.tile([P, G, DOUT], F32)

                # ---- stage A: XBAR-transpose skewed one tile ahead of the
                #      matmuls so the PE never waits on repartitioning ----
                xt_list = [None] * (G // 4)
                y2 = None
                for t in range(G + 4):
                    if t % 4 == 0 and t < G:
                        h = t // 4
                        xt4 = xtp.tile([P, 4, KC, P], FP16, name="xt4")
                        nc.sync.dma_start(
                            xt4, x_buf[:, t : t + 4, :], transpose=True
                        )
                        xt_list[h] = xt4
                    if t >= 4:
                        tt = t - 4
                        if tt % 2 == 0:
                            y2 = psy.tile([P, 2, DOUT], F32)
                        for k in range(KC):
                            nc.tensor.matmul(
                                y2[:, tt % 2, :],
                                xt_list[tt // 4][:, tt % 4, k, :],
                                wb_sb[:, k, :],
                                start=(k == 0),
                                stop=(k == KC - 1) and not use_bias,
                            )
                        if use_bias:
                            nc.tensor.matmul(
                                y2[:, tt % 2, :], ones_sb, bb_sb, start=False, stop=True
                            )
                        if tt % 2 == 1:
                            nc.vector.tensor_mul(
                                z_buf[:, tt - 1 : tt + 1, :],
                                y2,
                                p_buf[:, tt - 1 : tt + 1, :],
                            )
                            nc.vector.max(m8[:, tt - 1, :], z_buf[:, tt - 1, :])
                            nc.vector.max(m8[:, tt, :], z_buf[:, tt, :])

                # ---- stage B: tau0 from top-8 prefix (DVE + GPS) ----
                mflat = m8.rearrange("p g s -> p (g s)")
                cum = smp.tile([P, G * K8], F32, tag="cum")
                nc.vector.tensor_tensor_scan(
                    out=cum,
                    data0=keep_sb,
                    data1=mflat,
                    initial=0.0,
                    op0=Alu.mult,
                    op1=Alu.add,
                )
                jm = smp.tile([P, G * K8], F32, tag="jm")
                nc.gpsimd.tensor_mul(jm, mflat, iota_sb)
                cm1 = smp.tile([P, G * K8], F32, tag="cm1")
                nc.vector.tensor_scalar_sub(cm1, cum, 1.0)
                mask = smp.tile([P, G * K8], F32, tag="mask")
                nc.vector.tensor_tensor(out=mask, in0=jm, in1=cm1, op=Alu.is_gt)
                msel = smp.tile([P, G * K8], F32, tag="msel")
                nc.gpsimd.tensor_mul(msel, mflat, mask)

                s8 = smp.tile([P, G], F32, tag="s8")
                nc.vector.reduce_sum(
                    s8,
                    msel.rearrange("p (g s) -> p g s", s=K8),
                    axis=mybir.AxisListType.X,
                )
                k8 = smp.tile([P, G], F32, tag="k8")
                nc.vector.reduce_sum(
                    k8,
                    mask.rearrange("p (g s) -> p g s", s=K8),
                    axis=mybir.AxisListType.X,
                )
                kr = smp.tile([P, G], F32, tag="kr")
                nc.vector.reciprocal(kr, k8)
                tau0 = smp.tile([P, G], F32, tag="tau0")
                nc.vector.tensor_scalar(
                    out=tau0, in0=s8, scalar1=-1.0, scalar2=None, op0=Alu.add
                )
                nc.vector.tensor_mul(tau0, tau0, kr)
                ntau0 = smp.tile([P, G], F32, tag="ntau0")
                nc.vector.tensor_scalar_mul(ntau0, tau0, -1.0)
                # ---- stage E: out = relu(z - tau0) on ACT, batched store on
                #      the gpsimd queue (ahead of the next x prefetch) ----
                for t in range(G):
                    nc.scalar.activation(
                        out_buf[:, t, :],
                        z_buf[:, t, :],
                        Act.Relu,
                        bias=ntau0[:, t : t + 1],
                    )
                nc.scalar.dma_start(og[g], out_buf)

    nc.compile()
    return nc


def kernel(input_x, priors, W, bn_scale, bn_bias, bn_mean, bn_var):
    global LAST_RESULTS
    input_x = np.ascontiguousarray(input_x, dtype=np.float32)
    priors = np.ascontiguousarray(priors, dtype=np.float32)

    inv = (
        bn_scale.astype(np.float32)
        / np.sqrt(bn_var.astype(np.float32) + np.float32(BN_EPS))
    ).astype(np.float32)
    wf = np.ascontiguousarray(W.astype(np.float32) * inv[None, :])
    bf = np.ascontiguousarray(
        (bn_bias.astype(np.float32) - bn_mean.astype(np.float32) * inv)[None, :]
    )
    use_bias = bool(np.any(bf != 0.0))

    iota8 = np.ascontiguousarray(
        np.tile(np.arange(1, K8 + 1, dtype=np.float32), (P, G))
    )

    key = ("nc", use_bias)
    if key not in _CACHE:
        _CACHE[key] = _build(use_bias)
    nc = _CACHE[key]

    in_maps = []
    for c in range(NCORES):
        in_maps.append(
            {
                "x": input_x[c * BC : (c + 1) * BC],
                "priors": priors[c * BC : (c + 1) * BC],
                "w": wf,
                "b": bf,
                "iota8": iota8,
            }
        )

    res = run_bass_kernel_spmd(nc, in_maps, list(range(NCORES)))
    LAST_RESULTS = res
    out = np.concatenate([res.results[c]["out"] for c in range(NCORES)], axis=0)
    return out


# revision 28
# speedup vs baseline: 1.7728x; 1.6883x over previous
"""AttentiveTransformer (Dense + BN(eval) + prior-scale + sparsemax) on 8 TRN2 cores.

Math per row (B=131072 rows, data-parallel over 8 cores):
    y   = x @ (W * bn_inv) + (bn_bias - bn_mean * bn_inv)   # BN folded into W/bias
    z   = y * priors
    out = sparsemax(z)          # row-wise, D=256, support capped at top-8

Device pipeline per 128-row tile (engine-balanced):
    PE  : 4x transpose of x chunks (f32 identity matmul) + 4x fp16 matmul
          (fp16 halves the LDWEIGHTS stationary-load cost; rel-err measured
          identical to the f32r path at 2.5e-3)
    ACT : PSUM->SBUF copy of x^T (2 tiles per op, converts to fp16) and
          6 of 8 relu(z - tau0) activations
    DVE : z = y*priors (PSUM read, 2 tiles/op), top-8 via max8, prefix-scan
          tau0 math, 2 of 8 relus
    GPS : iota/msel muls, priors loads + out stores (software DGE queue)
    SYNC: chunked x loads (hardware DGE)

Sharding: pure data-parallel on the batch dim; W/BN replicated per core.
"""

import numpy as np

import concourse.mybir as mybir
import concourse.tile as tile
from concourse import bacc
from concourse.bass_utils import run_bass_kernel_spmd
from concourse.masks import make_identity

F32 = mybir.dt.float32
FP16 = mybir.dt.float16
Alu = mybir.AluOpType
Act = mybir.ActivationFunctionType

NCORES = 8
B = 131072
DIN = 512
DOUT = 256
P = 128
BC = B // NCORES            # rows per core (16384)
G = 8                       # row-tiles per super-batch
TILES = BC // P             # row-tiles per core (128)
NBATCH = TILES // G         # super-batches per core (16)
KC = DIN // P               # K chunks (4)
K8 = 8

BN_EPS = 1e-5

_CACHE = {}
LAST_RESULTS = None


def _build(use_bias):
    nc = bacc.Bacc("TRN2", target_bir_lowering=False, debug=False)

    x_d = nc.dram_tensor("x", [BC, DIN], F32, kind="ExternalInput").ap()
    pri_d = nc.dram_tensor("priors", [BC, DOUT], F32, kind="ExternalInput").ap()
    w_d = nc.dram_tensor("w", [DIN, DOUT], F32, kind="ExternalInput").ap()
    b_d = nc.dram_tensor("b", [1, DOUT], F32, kind="ExternalInput").ap()
    iota_d = nc.dram_tensor("iota8", [P, G * K8], F32, kind="ExternalInput").ap()
    out_d = nc.dram_tensor("out", [BC, DOUT], F32, kind="ExternalOutput").ap()

    xg = x_d.rearrange("(g p t) d -> g p t d", p=P, t=G)
    pg = pri_d.rearrange("(g p t) d -> g p t d", p=P, t=G)
    og = out_d.rearrange("(g p t) d -> g p t d", p=P, t=G)

    with tile.TileContext(nc) as tc:
        with (
            tc.tile_pool(name="static", bufs=1) as sp,
            tc.tile_pool(name="xin", bufs=3) as xp,
            tc.tile_pool(name="pin", bufs=3) as pp,
            tc.tile_pool(name="oout", bufs=3) as op_,
            tc.tile_pool(name="zb", bufs=3) as zp,
            tc.tile_pool(name="xt", bufs=6) as xtp,
            tc.tile_pool(name="small", bufs=3) as smp,
            tc.tile_pool(name="pst", bufs=2, space="PSUM") as pst,
            tc.tile_pool(name="psy", bufs=4, space="PSUM") as psy,
        ):
            # ---- statics (on the scalar HWDGE queue so the sync queue's
            #      head is the first x chunk) ----
            ident = sp.tile([P, P], F32)
            make_identity(nc, ident)

            w_sb = sp.tile([P, KC, DOUT], F32)
            nc.scalar.dma_start(w_sb, w_d.rearrange("(c p) n -> p c n", p=P))
            wh_sb = sp.tile([P, KC, DOUT], FP16)
            nc.vector.tensor_copy(wh_sb, w_sb)

            if use_bias:
                b_sb = sp.tile([1, DOUT], F32)
                nc.scalar.dma_start(b_sb, b_d)
                bh_sb = sp.tile([1, DOUT], FP16)
                nc.vector.tensor_copy(bh_sb, b_sb)
                ones_sb = sp.tile([1, P], FP16)
                nc.vector.memset(ones_sb, 1.0)

            iota_sb = sp.tile([P, G * K8], F32)
            nc.scalar.dma_start(iota_sb, iota_d)

            keep_sb = sp.tile([P, G * K8], F32)
            nc.vector.memset(keep_sb, 1.0)
            nc.vector.memset(
                keep_sb.rearrange("p (g s) -> p g s", s=K8)[:, :, 0:1], 0.0
            )

            x_bufs = {}
            p_bufs = {}

            def load_x(gi):
                x_bufs[gi] = xp.tile([P, G, DIN], F32, name="xbuf")
                xc = 2 if gi == 0 else 4
                for c in range(0, G, xc):
                    nc.sync.dma_start(
                        x_bufs[gi][:, c : c + xc, :], xg[gi][:, c : c + xc, :]
                    )

            def load_p(gi):
                p_bufs[gi] = pp.tile([P, G, DOUT], F32, name="pbuf")
                nc.gpsimd.dma_start(p_bufs[gi], pg[gi])

            for g in range(NBATCH):
                if g == 0:
                    load_x(0)
                    load_p(0)
                if g + 1 < NBATCH:
                    load_x(g + 1)
                    load_p(g + 1)
                x_buf = x_bufs.pop(g)
                p_buf = p_bufs.pop(g)

                z_buf = zp.tile([P, G, DOUT], F32)
                m8 = smp.tile([P, G, K8], F32, tag="m8")
                out_buf = op_.tile([P, G, DOUT], F32)

                # ---- stage A: software-skewed pipeline on PE:
                #      transposes of tile-pair h run before matmuls of pair
                #      h-1, so PE never head-of-line blocks on the ACT copy --
                xt_list = [None] * G
                y2 = None
                for t in range(G + 2):
                    if t < G and t % 2 == 0:
                        xt_ps = pst.tile([P, 2, DIN], F32)
                        for u in range(2):
                            for k in range(KC):
                                nc.tensor.transpose(
                                    xt_ps[:, u, k * P : (k + 1) * P],
                                    x_buf[:, t + u, k * P : (k + 1) * P],
                                    ident,
                                )
                        xt_sb = xtp.tile([P, 2, KC, P], FP16, name="xtsb")
                        nc.scalar.copy(
                            xt_sb,
                            xt_ps.rearrange("p u (c q) -> p u c q", c=KC),
                        )
                        xt_list[t] = xt_sb
                    if t >= 2:
                        tt = t - 2
                        if tt % 2 == 0:
                            y2 = psy.tile([P, 2, DOUT], F32)
                        for k in range(KC):
                            nc.tensor.matmul(
                                y2[:, tt % 2, :],
                                xt_list[tt - tt % 2][:, tt % 2, k, :],
                                wh_sb[:, k, :],
                                start=(k == 0),
                                stop=(k == KC - 1) and not use_bias,
                            )
                        if use_bias:
                            nc.tensor.matmul(
                                y2[:, tt % 2, :], ones_sb, bh_sb, start=False, stop=True
                            )
                        if tt % 2 == 1:
                            nc.vector.tensor_mul(
                                z_buf[:, tt - 1 : tt + 1, :],
                                y2,
                                p_buf[:, tt - 1 : tt + 1, :],
                            )
                            nc.vector.max(m8[:, tt - 1, :], z_buf[:, tt - 1, :])
                            nc.vector.max(m8[:, tt, :], z_buf[:, tt, :])

                # ---- stage B: tau0 from top-8 prefix (DVE + GPS) ----
                mflat = m8.rearrange("p g s -> p (g s)")
                cum = smp.tile([P, G * K8], F32, tag="cum")
                nc.vector.tensor_tensor_scan(
                    out=cum,
                    data0=keep_sb,
                    data1=mflat,
                    initial=0.0,
                    op0=Alu.mult,
                    op1=Alu.add,
                )
                jm = smp.tile([P, G * K8], F32, tag="jm")
                nc.gpsimd.tensor_mul(jm, mflat, iota_sb)
                cm1 = smp.tile([P, G * K8], F32, tag="cm1")
                nc.vector.tensor_scalar_sub(cm1, cum, 1.0)
                mask = smp.tile([P, G * K8], F32, tag="mask")
                nc.vector.tensor_tensor(out=mask, in0=jm, in1=cm1, op=Alu.is_gt)
                msel = smp.tile([P, G * K8], F32, tag="msel")
                nc.gpsimd.tensor_mul(msel, mflat, mask)

                s8 = smp.tile([P, G], F32, tag="s8")
                nc.vector.reduce_sum(
                    s8,
                    msel.rearrange("p (g s) -> p g s", s=K8),
                    axis=mybir.AxisListType.X,
                )
                k8 = smp.tile([P, G], F32, tag="k8")
                nc.vector.reduce_sum(
                    k8,
                    mask.rearrange("p (g s) -> p g s", s=K8),
                    axis=mybir.AxisListType.X,
                )
                kr = smp.tile([P, G], F32, tag="kr")
                nc.vector.reciprocal(kr, k8)
                tau0 = smp.tile([P, G], F32, tag="tau0")
                nc.vector.tensor_scalar(
                    out=tau0, in0=s8, scalar1=-1.0, scalar2=None, op0=Alu.add
                )
                nc.vector.tensor_mul(tau0, tau0, kr)
                ntau0 = smp.tile([P, G], F32, tag="ntau0")
                nc.vector.tensor_scalar_mul(ntau0, tau0, -1.0)
                # ---- stage E: out = relu(z - tau0), 6 tiles on ACT + 2 on
                #      DVE, one batched store on the gpsimd queue ----
                for t in range(G):
                    if t < 2:
                        nc.vector.tensor_scalar(
                            out=out_buf[:, t, :],
                            in0=z_buf[:, t, :],
                            scalar1=ntau0[:, t : t + 1],
                            scalar2=0.0,
                            op0=Alu.add,
                            op1=Alu.max,
                        )
                    else:
                        nc.scalar.activation(
                            out_buf[:, t, :],
                            z_buf[:, t, :],
                            Act.Relu,
                            bias=ntau0[:, t : t + 1],
                        )
                nc.gpsimd.dma_start(og[g], out_buf)

    nc.compile()
    return nc


def kernel(input_x, priors, W, bn_scale, bn_bias, bn_mean, bn_var):
    global LAST_RESULTS
    input_x = np.ascontiguousarray(input_x, dtype=np.float32)
    priors = np.ascontiguousarray(priors, dtype=np.float32)

    inv = (
        bn_scale.astype(np.float32)
        / np.sqrt(bn_var.astype(np.float32) + np.float32(BN_EPS))
    ).astype(np.float32)
    wf = np.ascontiguousarray(W.astype(np.float32) * inv[None, :])
    bf = np.ascontiguousarray(
        (bn_bias.astype(np.float32) - bn_mean.astype(np.float32) * inv)[None, :]
    )
    use_bias = bool(np.any(bf != 0.0))

    iota8 = np.ascontiguousarray(
        np.tile(np.arange(1, K8 + 1, dtype=np.float32), (P, G))
    )

    key = ("nc", use_bias)
    if key not in _CACHE:
        _CACHE[key] = _build(use_bias)
    nc = _CACHE[key]

    in_maps = []
    for c in range(NCORES):
        in_maps.append(
            {
                "x": input_x[c * BC : (c + 1) * BC],
                "priors": priors[c * BC : (c + 1) * BC],
                "w": wf,
                "b": bf,
                "iota8": iota8,
            }
        )

    res = run_bass_kernel_spmd(nc, in_maps, list(range(NCORES)))
    LAST_RESULTS = res
    out = np.concatenate([res.results[c]["out"] for c in range(NCORES)], axis=0)
    return out


# revision 31
# speedup vs baseline: 1.8305x; 1.0325x over previous
"""AttentiveTransformer (Dense + BN(eval) + prior-scale + sparsemax) on 8 TRN2 cores.

Math per row (B=131072 rows, data-parallel over 8 cores):
    y   = x @ (W * bn_inv) + (bn_bias - bn_mean * bn_inv)   # BN folded into W/bias
    z   = y * priors
    out = sparsemax(z)          # row-wise, D=256, support capped at top-8

Device pipeline per 128-row tile (engine-balanced):
    PE  : 4x transpose of x chunks (f32 identity matmul) + 4x fp16 matmul
          (fp16 halves the LDWEIGHTS stationary-load cost; rel-err measured
          identical to the f32r path at 2.5e-3)
    ACT : PSUM->SBUF copy of x^T (2 tiles per op, converts to fp16) and
          6 of 8 relu(z - tau0) activations
    DVE : z = y*priors (PSUM read, 2 tiles/op), top-8 via max8, prefix-scan
          tau0 math, 2 of 8 relus
    GPS : iota/msel muls, priors loads + out stores (software DGE queue)
    SYNC: chunked x loads (hardware DGE)

Sharding: pure data-parallel on the batch dim; W/BN replicated per core.
"""

import numpy as np

import concourse.mybir as mybir
import concourse.tile as tile
from concourse import bacc
from concourse.bass_utils import run_bass_kernel_spmd
from concourse.masks import make_identity

F32 = mybir.dt.float32
FP16 = mybir.dt.float16
Alu = mybir.AluOpType
Act = mybir.ActivationFunctionType

NCORES = 8
B = 131072
DIN = 512
DOUT = 256
P = 128
BC = B // NCORES            # rows per core (16384)
G = 8                       # row-tiles per super-batch
TILES = BC // P             # row-tiles per core (128)
NBATCH = TILES // G         # super-batches per core (16)
KC = DIN // P               # K chunks (4)
K8 = 8

BN_EPS = 1e-5

_CACHE = {}
LAST_RESULTS = None


def _build(use_bias):
    nc = bacc.Bacc("TRN2", target_bir_lowering=False, debug=False)

    x_d = nc.dram_tensor("x", [BC, DIN], F32, kind="ExternalInput").ap()
    pri_d = nc.dram_tensor("priors", [BC, DOUT], F32, kind="ExternalInput").ap()
    w_d = nc.dram_tensor("w", [DIN, DOUT], F32, kind="ExternalInput").ap()
    b_d = nc.dram_tensor("b", [1, DOUT], F32, kind="ExternalInput").ap()
    iota_d = nc.dram_tensor("iota8", [P, G * K8], F32, kind="ExternalInput").ap()
    out_d = nc.dram_tensor("out", [BC, DOUT], F32, kind="ExternalOutput").ap()

    xg = x_d.rearrange("(g p t) d -> g p t d", p=P, t=G)
    pg = pri_d.rearrange("(g p t) d -> g p t d", p=P, t=G)
    og = out_d.rearrange("(g p t) d -> g p t d", p=P, t=G)

    with tile.TileContext(nc) as tc:
        with (
            tc.tile_pool(name="static", bufs=1) as sp,
            tc.tile_pool(name="xin", bufs=3) as xp,
            tc.tile_pool(name="pin", bufs=3) as pp,
            tc.tile_pool(name="oout", bufs=3) as op_,
            tc.tile_pool(name="zb", bufs=3) as zp,
            tc.tile_pool(name="xt", bufs=6) as xtp,
            tc.tile_pool(name="small", bufs=3) as smp,
            tc.tile_pool(name="pst", bufs=3, space="PSUM") as pst,
            tc.tile_pool(name="psy", bufs=5, space="PSUM") as psy,
        ):
            # ---- statics (on the scalar HWDGE queue so the sync queue's
            #      head is the first x chunk) ----
            ident = sp.tile([P, P], F32)
            make_identity(nc, ident)

            w_sb = sp.tile([P, KC, DOUT], F32)
            nc.scalar.dma_start(w_sb, w_d.rearrange("(c p) n -> p c n", p=P))
            wh_sb = sp.tile([P, KC, DOUT], FP16)
            nc.vector.tensor_copy(wh_sb, w_sb)

            if use_bias:
                b_sb = sp.tile([1, DOUT], F32)
                nc.scalar.dma_start(b_sb, b_d)
                bh_sb = sp.tile([1, DOUT], FP16)
                nc.vector.tensor_copy(bh_sb, b_sb)
                ones_sb = sp.tile([1, P], FP16)
                nc.vector.memset(ones_sb, 1.0)

            iota_sb = sp.tile([P, G * K8], F32)
            nc.scalar.dma_start(iota_sb, iota_d)

            keep_sb = sp.tile([P, G * K8], F32)
            nc.vector.memset(keep_sb, 1.0)
            nc.vector.memset(
                keep_sb.rearrange("p (g s) -> p g s", s=K8)[:, :, 0:1], 0.0
            )

            x_bufs = {}
            p_bufs = {}

            def load_x(gi):
                x_bufs[gi] = xp.tile([P, G, DIN], F32, name="xbuf")
                xc = 2 if gi == 0 else 4
                for c in range(0, G, xc):
                    nc.sync.dma_start(
                        x_bufs[gi][:, c : c + xc, :], xg[gi][:, c : c + xc, :]
                    )

            def load_p(gi):
                p_bufs[gi] = pp.tile([P, G, DOUT], F32, name="pbuf")
                nc.gpsimd.dma_start(p_bufs[gi], pg[gi])

            for g in range(NBATCH):
                if g == 0:
                    load_x(0)
                    load_p(0)
                if g + 1 < NBATCH:
                    load_x(g + 1)
                    load_p(g + 1)
                x_buf = x_bufs.pop(g)
                p_buf = p_bufs.pop(g)

                z_buf = zp.tile([P, G, DOUT], F32)
                m8 = smp.tile([P, G, K8], F32, tag="m8")
                out_buf = op_.tile([P, G, DOUT], F32)

                # ---- stage A: software-skewed pipeline on PE:
                #      transposes of tile-pair h run before matmuls of pair
                #      h-1, so PE never head-of-line blocks on the ACT copy --
                xt_list = [None] * G
                y2 = None
                for t in range(G + 1):
                    if t < G:
                        xt_ps = pst.tile([P, DIN], F32)
                        for k in range(KC):
                            nc.tensor.transpose(
                                xt_ps[:, k * P : (k + 1) * P],
                                x_buf[:, t, k * P : (k + 1) * P],
                                ident,
                            )
                        xt_sb = xtp.tile([P, KC, P], FP16, name="xtsb")
                        nc.scalar.copy(
                            xt_sb, xt_ps.rearrange("p (c q) -> p c q", c=KC)
                        )
                        xt_list[t] = xt_sb
                    if t >= 1:
                        tt = t - 1
                        if tt % 2 == 0:
                            y2 = psy.tile([P, 2, DOUT], F32)
                        for k in range(KC):
                            nc.tensor.matmul(
                                y2[:, tt % 2, :],
                                xt_list[tt][:, k, :],
                                wh_sb[:, k, :],
                                start=(k == 0),
                                stop=(k == KC - 1) and not use_bias,
                            )
                        if use_bias:
                            nc.tensor.matmul(
                                y2[:, tt % 2, :], ones_sb, bh_sb, start=False, stop=True
                            )
                        if tt % 2 == 1:
                            nc.vector.tensor_mul(
                                z_buf[:, tt - 1 : tt + 1, :],
                                y2,
                                p_buf[:, tt - 1 : tt + 1, :],
                            )
                            nc.vector.max(m8[:, tt - 1, :], z_buf[:, tt - 1, :])
                            nc.vector.max(m8[:, tt, :], z_buf[:, tt, :])

                # ---- stage B: tau0 from top-8 prefix (DVE + GPS) ----
                mflat = m8.rearrange("p g s -> p (g s)")
                cum = smp.tile([P, G * K8], F32, tag="cum")
                nc.vector.tensor_tensor_scan(
                    out=cum,
                    data0=keep_sb,
                    data1=mflat,
                    initial=0.0,
                    op0=Alu.mult,
                    op1=Alu.add,
                )
                jm = smp.tile([P, G * K8], F32, tag="jm")
                nc.gpsimd.tensor_mul(jm, mflat, iota_sb)
                cm1 = smp.tile([P, G * K8], F32, tag="cm1")
                nc.vector.tensor_scalar_sub(cm1, cum, 1.0)
                mask = smp.tile([P, G * K8], F32, tag="mask")
                nc.vector.tensor_tensor(out=mask, in0=jm, in1=cm1, op=Alu.is_gt)
                msel = smp.tile([P, G * K8], F32, tag="msel")
                nc.gpsimd.tensor_mul(msel, mflat, mask)

                s8 = smp.tile([P, G], F32, tag="s8")
                nc.vector.reduce_sum(
                    s8,
                    msel.rearrange("p (g s) -> p g s", s=K8),
                    axis=mybir.AxisListType.X,
                )
                k8 = smp.tile([P, G], F32, tag="k8")
                nc.vector.reduce_sum(
                    k8,
                    mask.rearrange("p (g s) -> p g s", s=K8),
                    axis=mybir.AxisListType.X,
                )
                kr = smp.tile([P, G], F32, tag="kr")
                nc.vector.reciprocal(kr, k8)
                tau0 = smp.tile([P, G], F32, tag="tau0")
                nc.vector.tensor_scalar(
                    out=tau0, in0=s8, scalar1=-1.0, scalar2=None, op0=Alu.add
                )
                nc.vector.tensor_mul(tau0, tau0, kr)
                ntau0 = smp.tile([P, G], F32, tag="ntau0")
                nc.vector.tensor_scalar_mul(ntau0, tau0, -1.0)
                # ---- stage E: out = relu(z - tau0), 6 tiles on ACT + 2 on
                #      DVE, one batched store on the gpsimd queue ----
                for t in range(G):
                    if t < 2:
                        nc.vector.tensor_scalar(
                            out=out_buf[:, t, :],
                            in0=z_buf[:, t, :],
                            scalar1=ntau0[:, t : t + 1],
                            scalar2=0.0,
                            op0=Alu.add,
                            op1=Alu.max,
                        )
                    else:
                        nc.scalar.activation(
                            out_buf[:, t, :],
                            z_buf[:, t, :],
                            Act.Relu,
                            bias=ntau0[:, t : t + 1],
                        )
                    if t % 4 == 3:
                        nc.gpsimd.dma_start(
                            og[g][:, t - 3 : t + 1, :], out_buf[:, t - 3 : t + 1, :]
                        )

    nc.compile()
    return nc


def kernel(input_x, priors, W, bn_scale, bn_bias, bn_mean, bn_var):
    global LAST_RESULTS
    input_x = np.ascontiguousarray(input_x, dtype=np.float32)
    priors = np.ascontiguousarray(priors, dtype=np.float32)

    inv = (
        bn_scale.astype(np.float32)
        / np.sqrt(bn_var.astype(np.float32) + np.float32(BN_EPS))
    ).astype(np.float32)
    wf = np.ascontiguousarray(W.astype(np.float32) * inv[None, :])
    bf = np.ascontiguousarray(
        (bn_bias.astype(np.float32) - bn_mean.astype(np.float32) * inv)[None, :]
    )
    use_bias = bool(np.any(bf != 0.0))

    iota8 = np.ascontiguousarray(
        np.tile(np.arange(1, K8 + 1, dtype=np.float32), (P, G))
    )

    key = ("nc", use_bias)
    if key not in _CACHE:
        _CACHE[key] = _build(use_bias)
    nc = _CACHE[key]

    in_maps = []
    for c in range(NCORES):
        in_maps.append(
            {
                "x": input_x[c * BC : (c + 1) * BC],
                "priors": priors[c * BC : (c + 1) * BC],
                "w": wf,
                "b": bf,
                "iota8": iota8,
            }
        )

    res = run_bass_kernel_spmd(nc, in_maps, list(range(NCORES)))
    LAST_RESULTS = res
    out = np.concatenate([res.results[c]["out"] for c in range(NCORES)], axis=0)
    return out
